# revision 74
# baseline (speedup 1.0000x reference)
"""Multi-head attention (B=2, M=N=2048, D=1024, H=16, DH=64) on 8 TRN2 cores.

Sharding: data-parallel over batch (cores 0-3 = batch 0, 4-7 = batch 1),
tensor-parallel over heads within each batch group (4 heads/core).

Engine plan (per core):
  PE      : x transposes (bf16-identity moving tensor => 1 cyc/row, data
            stays exact f32r), Q/K/V projections (f32r, F>=256), S^T
            matmuls (f32r, F=512), AV in O-natural orientation (bf16
            at/v2, F=65 incl. ones column for denominators), O^T
            transposes, out-projection (bf16).
  Act     : exp ONLY (the S^T->exp->AV chain is Act-limited; everything
            else is moved off this engine).
  DVE     : PSUM->SBUF copies, biases, reciprocal + per-query normalize.
  Pool    : half the stage-1 copies + weight DMAs.
  DMA     : chunked x loads, per-m bf16 AllGather feed, out stores.

Emission order = per-engine execution order (in-order engines), so the
program interleaves: K/Q chunk transposes+projections feed the first
S^T quanta at ~10us; a fill queue (V work, AV quanta, normalize, O^T
transpose + AllGather, out-projection) pops between S^T quanta to keep
PE busy while Act drains exps. Each m-chunk's (256,512) O^T shard
AllGathers (bf16) immediately so the out-projection pipelines under
later chunks' attention.

AV orientation: O natural [q partitions, d free]: lhsT = exp(S^T) tile
(keys on partitions), rhs = [V | ones] so column DH accumulates the
softmax denominator; normalization is then a per-partition DVE
tensor_scalar multiply (no gpsimd broadcast needed).

Host-side prep identical to the baseline: per-core transposed/sliced
weights, bv folded into bo_eff = bo + Wo @ bv, bk dropped (softmax
cancels row-constant logit shifts). Output assembly concatenates
per-core (256, 2048) out^T slices.
"""

import os

import numpy as np

B, M, NSEQ, D = 2, 2048, 2048, 1024
H, DH = 16, 64
HC = 4                # heads per core
PC = HC * DH          # 256 projected channels per core
CT = D // 128         # 8 contraction tiles
NT = NSEQ // 128      # 16 n-tiles
MT = M // 512         # 4 m-chunks
NCORES = 8

AT_BUFS = 12          # in-flight exp(S^T) tiles (bf16, 2KB/partition each)
FILL_NS = 1150.0      # non-S^T PE ns to interleave per S^T quantum

_CACHE = {}


def _build(single_core=False, reps=1):
    import concourse.bass as bass
    import concourse.tile as tile
    from concourse import bacc, mybir
    from concourse.masks import make_identity

    F32 = mybir.dt.float32
    F32R = mybir.dt.float32r
    BF16 = mybir.dt.bfloat16
    AF = mybir.ActivationFunctionType

    nc = bacc.Bacc(
        "TRN2",
        target_bir_lowering=False,
        debug=False,
        num_devices=1 if single_core else 8,
    )

    xq_d = nc.dram_tensor("xq", [M, D], F32, kind="ExternalInput")
    xk_d = nc.dram_tensor("xk", [NSEQ, D], F32, kind="ExternalInput")
    xv_d = nc.dram_tensor("xv", [NSEQ, D], F32, kind="ExternalInput")
    wqT_d = nc.dram_tensor("wqT", [D, PC], F32, kind="ExternalInput")
    wkT_d = nc.dram_tensor("wkT", [D, PC], F32, kind="ExternalInput")
    wvT_d = nc.dram_tensor("wvT", [D, PC], F32, kind="ExternalInput")
    woT_d = nc.dram_tensor("woT", [D, PC], F32, kind="ExternalInput")
    bq_d = nc.dram_tensor("bq", [PC, 1], F32, kind="ExternalInput")
    bo_d = nc.dram_tensor("bo", [PC, 1], F32, kind="ExternalInput")
    outT_d = nc.dram_tensor("outT", [PC, M], F32, kind="ExternalOutput")
    debug = bool(int(os.environ.get("KERNEL_DEBUG", "0")))
    dbg = None
    if debug:
        dbg = dict(
            kT=nc.dram_tensor("kT_dbg", [128, 2, NSEQ], F32, kind="ExternalOutput"),
            qT=nc.dram_tensor("qT_dbg", [128, 2, M], F32, kind="ExternalOutput"),
            v2=nc.dram_tensor(
                "v2_dbg", [128, HC, NT, DH + 1], F32, kind="ExternalOutput"
            ),
            agin=nc.dram_tensor("agin_dbg", [MT, PC, 512], F32, kind="ExternalOutput"),
            agout=nc.dram_tensor(
                "agout_dbg", [MT, 4 * PC, 512], F32, kind="ExternalOutput"
            ),
            at0=nc.dram_tensor("at0_dbg", [128, 1024], F32, kind="ExternalOutput"),
            osb=nc.dram_tensor(
                "osb_dbg", [MT, 128, 4, PC], F32, kind="ExternalOutput"
            ),
        )

    with tile.TileContext(nc) as tc:
        with (
            tc.tile_pool(name="singles", bufs=1) as singles,
            tc.tile_pool(name="dram", bufs=1, space="DRAM") as dram,
        ):
            ident = singles.tile([128, 128], BF16)
            make_identity(nc, ident)
            # PE matmuls cannot mix 32-bit and 16-bit operands, and f32r
            # matmul inputs must come from a rounding producer — so the x
            # transposes stay f32 (the PSUM->SBUF copy rounds to f32r)
            ident_r = singles.tile([128, 128], F32)
            make_identity(nc, ident_r)
            bq_sb = singles.tile([128, 2], F32)
            nc.sync.dma_start(
                out=bq_sb, in_=bq_d[:, :].rearrange("(o p) w -> p (o w)", p=128)
            )
            bo_sb = singles.tile([128, 2], F32)
            nc.sync.dma_start(
                out=bo_sb, in_=bo_d[:, :].rearrange("(o p) w -> p (o w)", p=128)
            )

            ag_in = dram.tile([MT, PC, 512], BF16)
            # split gather: half h covers the two heads of pair h (channel
            # rows h*128..h*128+127 of each core's O^T shard), gathered
            # across the 4-core group as [cc, 128, 512]
            ag_out = dram.tile([MT, 2, 4, 128, 512], BF16)

            for rep in range(reps):
                _emit_rep(
                    nc, tc, bass, mybir, F32, F32R, BF16, AF, rep, single_core,
                    dict(
                        xq_d=xq_d, xk_d=xk_d, xv_d=xv_d, wqT_d=wqT_d,
                        wkT_d=wkT_d, wvT_d=wvT_d, woT_d=woT_d, outT_d=outT_d,
                        ident=ident, ident_r=ident_r, bq_sb=bq_sb,
                        bo_sb=bo_sb, ag_in=ag_in, ag_out=ag_out,
                        dbg=dbg if rep == reps - 1 else None,
                    ),
                )
    nc.compile()
    return nc


def _emit_rep(nc, tc, bass, mybir, F32, F32R, BF16, AF, rep, single_core, env):
    ident = env["ident"]
    ident_r = env["ident_r"]
    bq_sb, bo_sb = env["bq_sb"], env["bo_sb"]
    at_bufs_n = 8 if os.environ.get("KERNEL_DEBUG") == "1" else AT_BUFS
    ag_in, ag_out = env["ag_in"], env["ag_out"]
    R = f"r{rep}_"

    with (
        tc.tile_pool(name=f"{R}w", bufs=1) as w_pool,
        tc.tile_pool(name=f"{R}nat", bufs=9) as nat_pool,
        tc.tile_pool(name=f"{R}xT", bufs=2) as xT_pool,
        tc.tile_pool(name=f"{R}proj", bufs=1) as proj_pool,
        tc.tile_pool(name=f"{R}at", bufs=at_bufs_n) as at_pool,
        tc.tile_pool(
            name=f"{R}osb",
            bufs=1 if os.environ.get("KERNEL_DEBUG") == "1" else 2,
        ) as osb_pool,
        tc.tile_pool(
            name=f"{R}og",
            bufs=1 if os.environ.get("KERNEL_DEBUG") == "1" else 2,
        ) as og_pool,
        tc.tile_pool(name=f"{R}o", bufs=2) as o_pool,
        tc.tile_pool(name=f"{R}rec", bufs=4) as rec_pool,
        tc.tile_pool(name=f"{R}ps1", bufs=2, space="PSUM") as ps1,
        tc.tile_pool(name=f"{R}pss", bufs=2, space="PSUM") as ps_s_pool,
        tc.tile_pool(name=f"{R}pso", bufs=2, space="PSUM") as ps_o_pool,
    ):
        # ---- persistent per-rep tiles ----
        kT = proj_pool.tile([128, 2, NSEQ], F32R, name=f"{R}kT")
        qT = proj_pool.tile([128, 2, M], F32R, name=f"{R}qT")
        # AV rhs: [k(128), h, nt, 0:64]=V bf16, [..., 64]=ones (denominator)
        v2 = proj_pool.tile([128, HC, NT, DH + 1], BF16, name=f"{R}v2")
        nc.vector.memset(v2[:, :, :, DH : DH + 1], 1.0)

        xd = {"k": env["xk_d"], "q": env["xq_d"], "v": env["xv_d"]}

        # ---- emit helpers ----
        def psum_copy(dst, src):
            # all PSUM->SBUF copies on DVE: GPSIMD has no PSUM access and
            # Act is reserved for the exp stream
            nc.vector.tensor_copy(dst, src)

        # x chunk loads: 4 per-row-block DMAs per (tensor, chunk), issued in
        # expected consumption order, ~2 chunks ahead of the transposes
        LOADS = [("q", 0), ("k", 0), ("k", 1), ("v", 0), ("k", 2), ("v", 1),
                 ("k", 3), ("v", 2), ("v", 3), ("q", 1), ("q", 2), ("q", 3)]
        load_idx = [0]
        nat = {}

        def load_chunk(t, c):
            if (t, c) in nat:
                return
            tiles = []
            for i in range(4):
                nt_t = nat_pool.tile(
                    [128, D], F32, tag="nat", name=f"{R}nat_{t}{c}_{i}"
                )
                nc.sync.dma_start(
                    out=nt_t,
                    in_=xd[t][c * 512 + i * 128 : c * 512 + (i + 1) * 128, :],
                )
                tiles.append(nt_t)
            nat[(t, c)] = tiles

        def load_next():
            while load_idx[0] < len(LOADS) and LOADS[load_idx[0]] in nat:
                load_idx[0] += 1
            if load_idx[0] < len(LOADS):
                load_chunk(*LOADS[load_idx[0]])
                load_idx[0] += 1

        xT = {}

        def tr_tile(t, c, i):
            # transpose one 128-row tile of chunk c across all 8 ct blocks;
            # per-tile granularity lets the PE start as soon as one DMA
            # lands instead of waiting for the whole 512-row chunk
            if i == 0:
                load_chunk(t, c)  # JIT fallback; normally already loaded
                load_next()       # keep prefetch lead
                x_t = xT_pool.tile(
                    [128, CT, 512], F32R, tag="xT", name=f"{R}xT_{t}{c}"
                )
                xT[(t, c)] = x_t
            x_t = xT[(t, c)]
            tile_i = nat[(t, c)][i]
            for ch in range(2):
                pst = ps1.tile(
                    [128, 512], F32, tag="ps1", name=f"{R}pst{t}{c}_{i}_{ch}"
                )
                for k in range(4):
                    ct = ch * 4 + k
                    nc.tensor.transpose(
                        pst[:, k * 128 : (k + 1) * 128],
                        tile_i[:, ct * 128 : (ct + 1) * 128],
                        ident_r,
                    )
                psum_copy(
                    x_t[:, ch * 4 : (ch + 1) * 4, i * 128 : (i + 1) * 128],
                    pst[:, :].rearrange("p (k f) -> p k f", k=4),
                )

        def tr_chunk(t, c, half):
            for i in (2 * half, 2 * half + 1):
                tr_tile(t, c, i)

        def kq_proj(t, c, ot):
            dst_T = kT if t == "k" else qT
            pj = ps1.tile([128, 512], F32, tag="ps1", name=f"{R}pj{t}{c}_{ot}")
            for ct in range(CT):
                nc.tensor.matmul(
                    pj,
                    w_sb[t][:, ct, ot * 128 : (ot + 1) * 128],
                    xT[(t, c)][:, ct, :],
                    start=(ct == 0),
                    stop=(ct == CT - 1),
                )
            dst = dst_T[:, ot, c * 512 : (c + 1) * 512]
            if t == "q":
                nc.vector.tensor_scalar_add(dst, pj, bq_sb[:, ot : ot + 1])
            else:
                psum_copy(dst, pj)

        vproj_done = [0]

        def v_proj(ntile):
            c = ntile // 4
            psv = ps1.tile([128, PC], F32, tag="ps1", name=f"{R}psv{ntile}")
            for ct in range(CT):
                nc.tensor.matmul(
                    psv,
                    xT[("v", c)][:, ct, ntile % 4 * 128 : (ntile % 4 + 1) * 128],
                    w_sb["v"][:, ct, :],
                    start=(ct == 0),
                    stop=(ct == CT - 1),
                )
            nc.vector.tensor_copy(
                v2[:, :, ntile, 0:DH],
                psv[:, :].rearrange("p (h d) -> p h d", h=HC),
            )
            vproj_done[0] = ntile + 1

        at_tiles = {}
        pso = {}

        def st_quantum(m, p, ntile):
            ps_s = ps_s_pool.tile(
                [128, 1024], F32, tag="pss", name=f"{R}pss{m}_{p}_{ntile}"
            )
            for j in range(2):
                base = j * 64
                nc.tensor.matmul(
                    ps_s[:, j * 512 : (j + 1) * 512],
                    kT[base : base + 64, p, ntile * 128 : (ntile + 1) * 128],
                    qT[base : base + 64, p, m * 512 : (m + 1) * 512],
                    start=True,
                    stop=True,
                )
            at = at_pool.tile(
                [128, 1024], BF16, tag="at", name=f"{R}at{m}_{p}_{ntile}"
            )
            nc.scalar.activation(at, ps_s, AF.Exp)
            at_tiles[(m, p, ntile)] = at
            if env.get("dbg") and (m, p, ntile) == (0, 0, 0):
                atf = proj_pool.tile([128, 1024], F32, name=f"{R}atf")
                nc.vector.tensor_copy(atf, at)
                nc.sync.dma_start(out=env["dbg"]["at0"][:, :], in_=atf)

        def av_quantum(m, p, ntile):
            if ntile == 0:
                for j in range(2):
                    # [128, 4, 128] = exactly one 2KB PSUM bank per
                    # accumulator — avoids two accumulation groups sharing
                    # a bank (start=True behavior is then self-contained)
                    pso[(m, p, j)] = ps_o_pool.tile(
                        [128, 4, 128], F32, tag="pso",
                        name=f"{R}pso{m}_{p}_{j}",
                    )
            at = at_tiles.pop((m, p, ntile))
            for j in range(2):
                for qt in range(4):
                    # start=True zeroes the WHOLE 2KB PSUM bank, so only
                    # the very first matmul of the bank's accumulation
                    # (qt==0, nt==0) may set it; the other qt regions then
                    # accumulate from the zeroed state
                    nc.tensor.matmul(
                        pso[(m, p, j)][:, qt, 0 : DH + 1],
                        at[:, j * 512 + qt * 128 : j * 512 + (qt + 1) * 128],
                        v2[:, 2 * p + j, ntile, :],
                        start=(ntile == 0 and qt == 0),
                        stop=(ntile == NT - 1 and qt == 3),
                        skip_group_check=True,
                    )

        o_sb = {}

        def norm(m, p):
            if p == 0:
                o_sb[m] = o_pool.tile([128, 4, PC], F32, tag="o", name=f"{R}o{m}")
            for j in range(2):
                h = 2 * p + j
                ps = pso.pop((m, p, j))
                rec = rec_pool.tile(
                    [128, 4, 1], F32, tag="rec", name=f"{R}rec{m}_{h}"
                )
                nc.vector.reciprocal(rec, ps[:, :, DH : DH + 1])
                for qt in range(4):
                    nc.vector.tensor_scalar_mul(
                        o_sb[m][:, qt, h * DH : (h + 1) * DH],
                        ps[:, qt, 0:DH],
                        rec[:, qt, 0:1],
                    )

        def otr_ag(m, half):
            # transpose + gather ONE head-pair's 128-channel half of O^T:
            # half 0 (heads 2p=0,1, channels 0..127) launches right after
            # norm(m, p=0), ~half an m-block before half 1 — so the
            # AllGather latency hides under the remaining S^T stream.
            # f32 transposes (PSUM banks are f32-native); the SBUF copy
            # casts to bf16 for the AllGather.
            otr = ps_s_pool.tile(
                [128, 512], F32, tag="pss", name=f"{R}otr{m}_{half}"
            )
            for qt in range(4):
                nc.tensor.transpose(
                    otr[:, qt * 128 : (qt + 1) * 128],
                    o_sb[m][:, qt, half * 128 : (half + 1) * 128],
                    ident_r,
                )
            otr_sb = o_pool.tile(
                [128, 512], BF16, tag="otrsb", bufs=2, name=f"{R}otrsb{m}_{half}"
            )
            nc.vector.tensor_copy(otr_sb, otr)
            if env.get("dbg") and half == 1:
                nc.sync.dma_start(
                    out=env["dbg"]["osb"][m, :, :, :], in_=o_sb[m]
                )
            # scalar-engine DMA queue: keeps the gather path ordered among
            # itself but independent of the (long) nat-load sync queue
            nc.scalar.dma_start(
                out=ag_in[m, half * 128 : (half + 1) * 128, :].rearrange(
                    "(c p) f -> p c f", p=128
                ),
                in_=otr_sb,
            )
            if single_core:
                for rr in range(4):
                    nc.scalar.dma_start(
                        out=ag_out[m, half, rr, :, :],
                        in_=ag_in[m, half * 128 : (half + 1) * 128, :],
                    )
            else:
                nc.gpsimd.collective_compute(
                    "AllGather",
                    bass.mybir.AluOpType.bypass,
                    replica_groups=[[0, 1, 2, 3], [4, 5, 6, 7]],
                    ins=[ag_in[m, half * 128 : (half + 1) * 128, :].opt()],
                    outs=[ag_out[m, half, :, :, :].opt()],
                )

        og = {}
        osb = {}

        def og_load(m):
            # og ct blocks: gathered channel block ct = cc*2 + half
            # (core cc's heads, pair half) -> og[:, ct, :]
            og[m] = og_pool.tile([128, CT, 512], BF16, tag="og", name=f"{R}og{m}")
            for half in range(2):
                nc.scalar.dma_start(
                    out=og[m][:, :, :].rearrange(
                        "p (cc hh) f -> p hh cc f", hh=2
                    )[:, half, :, :],
                    in_=ag_out[m, half, :, :, :].rearrange(
                        "cc p f -> p cc f"
                    ),
                )
            osb[m] = osb_pool.tile([128, 2, 512], F32, tag="osb", name=f"{R}osb{m}")

        def out_proj(m, ot):
            po = ps_s_pool.tile([128, 512], F32, tag="pss", name=f"{R}po{m}_{ot}")
            for ct in range(CT):
                nc.tensor.matmul(
                    po,
                    wo_bf[:, ct, ot * 128 : (ot + 1) * 128],
                    og[m][:, ct, :],
                    start=(ct == 0),
                    stop=(ct == CT - 1),
                )
            nc.vector.tensor_scalar_add(
                osb[m][:, ot, :], po, bo_sb[:, ot : ot + 1]
            )
            if ot == 1:
                nc.scalar.dma_start(
                    out=env["outT_d"][:, m * 512 : (m + 1) * 512].rearrange(
                        "(o p) f -> p o f", p=128
                    ),
                    in_=osb[m],
                )
                og.pop(m)
                osb.pop(m)

        # ---- fill-queue scheduler ----
        emit_log = []
        log_on = bool(int(os.environ.get("KERNEL_EMIT_LOG", "0")))
        fill = []
        fill_hi = []  # popped first: Q chains (unblock the next m's S^T)
        deferred = []  # (ready_slot, item): held until slot counter passes
        slot = [0]
        deficit = [0.0]

        def release_deferred():
            while deferred and deferred[0][0] <= slot[0]:
                fill.append(deferred.pop(0)[1])

        def pop_one():
            item = (fill_hi or fill).pop(0)
            if len(item) == 3:
                ns, fn, label = item
            else:
                (ns, fn), label = item, getattr(item[1], "__name__", "?")
            if log_on:
                emit_log.append(f"pop:{label}")
            fn()
            return ns

        def pop_fill(budget):
            deficit[0] = min(deficit[0] + budget, 4 * FILL_NS)
            while (fill or fill_hi) and deficit[0] > 0:
                deficit[0] -= pop_one()

        kq_emitted = set()

        def ensure_kq(t, c):
            if (t, c) in kq_emitted:
                return
            kq_emitted.add((t, c))
            tr_chunk(t, c, 0)
            tr_chunk(t, c, 1)
            kq_proj(t, c, 0)
            kq_proj(t, c, 1)

        # prefetch the first two chunks (q0 for Qproj(m0), k0 for S^T nt0)
        # BEFORE the weight DMAs — the first transposes are the critical
        # path; weights aren't read until the first projection (~7us in)
        load_next()
        load_next()

        w_sb = {}
        for tname, wd in (("k", "wkT_d"), ("q", "wqT_d"), ("v", "wvT_d")):
            w_sb[tname] = w_pool.tile([128, CT, PC], F32R, name=f"{R}w{tname}")
            nc.gpsimd.dma_start(
                out=w_sb[tname],
                in_=env[wd][:, :].rearrange("(ct p) c -> p ct c", p=128),
            )
        wo_f32 = nat_pool.tile(
            [128, CT, PC], F32, tag="wof", bufs=1, name=f"{R}wof"
        )
        nc.gpsimd.dma_start(
            out=wo_f32,
            in_=env["woT_d"][:, :].rearrange("(ct p) c -> p ct c", p=128),
        )
        wo_bf = w_pool.tile([128, CT, PC], BF16, name=f"{R}wob")
        nc.vector.tensor_copy(wo_bf, wo_f32)

        v_enqueued = [False]

        def enqueue_v_chain():
            if v_enqueued[0]:
                return
            v_enqueued[0] = True
            for c in range(4):
                fill.append(
                    (900.0, lambda c=c: tr_chunk("v", c, 0), f"Vtr0:{c}")
                )
                fill.append(
                    (900.0, lambda c=c: tr_chunk("v", c, 1), f"Vtr1:{c}")
                )
                for k in range(4):
                    fill.append(
                        (900.0, lambda n=c * 4 + k: v_proj(n),
                         f"Vproj:{c * 4 + k}")
                    )

        av_pending = []

        def flush_av():
            while av_pending and vproj_done[0] > av_pending[0][2]:
                mm, pp, ntile = av_pending.pop(0)
                fill.append(
                    (450.0, lambda a=mm, b=pp, c=ntile: av_quantum(a, b, c),
                     f"AV:{mm}{pp}.{ntile}")
                )

        # ---- main S^T driver ----
        for m in range(MT):
            # finish any in-flight pre-emitted Q chain for this m first
            while m > 0 and ("q", m) not in kq_emitted and (fill or fill_hi):
                flush_av()
                pop_one()
            ensure_kq("q", m)
            for p in range(2):
                for ntile in range(NT):
                    ensure_kq("k", ntile // 4)
                    if m == 0 and p == 0 and ntile == 4:
                        enqueue_v_chain()
                    if m < MT - 1 and p == 1 and ntile == 4:
                        # pre-emit next m's Q chain (high priority) so the
                        # m boundary doesn't stall the S^T/exp stream
                        nm = m + 1
                        fill_hi.append(
                            (900.0, lambda c=nm: tr_chunk("q", c, 0),
                             f"Qtr0:{nm}")
                        )
                        fill_hi.append(
                            (900.0, lambda c=nm: tr_chunk("q", c, 1),
                             f"Qtr1:{nm}")
                        )
                        fill_hi.append(
                            (900.0, lambda c=nm: kq_proj("q", c, 0),
                             f"Qproj0:{nm}")
                        )

                        def q_done(c=nm):
                            kq_proj("q", c, 1)
                            kq_emitted.add(("q", c))

                        fill_hi.append((900.0, q_done, f"Qproj1:{nm}"))
                    # at-pool headroom: force-drain fill before exp would
                    # block the Act engine on a full at pool (deadlock via
                    # the in-order PE stream otherwise)
                    while len(at_tiles) >= at_bufs_n - 1 and (fill or fill_hi):
                        flush_av()
                        pop_one()
                    if log_on:
                        emit_log.append(f"ST:{m}{p}.{ntile}")
                    st_quantum(m, p, ntile)
                    slot[0] += 1
                    release_deferred()
                    av_pending.append((m, p, ntile))
                    flush_av()
                    pop_fill(FILL_NS)

                def block_tail(m=m, p=p):
                    norm(m, p)
                    otr_ag(m, p)
                    if p == 1:
                        og_load(m)
                        # hold the out-projection back ~20 S^T slots so the
                        # in-order PE stream doesn't hit it before the
                        # AllGather + og DMA have landed
                        deferred.append(
                            (slot[0] + 20,
                             (1750.0, lambda: out_proj(m, 0), f"oproj0:{m}"))
                        )
                        deferred.append(
                            (slot[0] + 20,
                             (1750.0, lambda: out_proj(m, 1), f"oproj1:{m}"))
                        )
                fill.append((500.0, block_tail, f"tail:{m}{p}"))

        # drain
        while av_pending or fill or fill_hi or deferred:
            slot[0] += 10_000
            release_deferred()
            flush_av()
            if fill or fill_hi:
                pop_one()
            else:
                assert not av_pending, "AV stuck without V projections"
        if log_on:
            print("EMIT ORDER:", " ".join(emit_log))

        if env.get("dbg"):
            d = env["dbg"]
            nc.sync.dma_start(out=d["kT"][:, :, :], in_=kT.bitcast(F32))
            nc.sync.dma_start(out=d["qT"][:, :, :], in_=qT.bitcast(F32))
            with tc.tile_pool(name=f"{R}dbgp", bufs=2) as dbgp:
                for h in range(HC):
                    vdump = dbgp.tile(
                        [128, NT, DH + 1], F32, tag="vd", bufs=1,
                        name=f"{R}vdump{h}",
                    )
                    nc.vector.tensor_copy(vdump, v2[:, h, :, :])
                    nc.sync.dma_start(out=d["v2"][:, h, :, :], in_=vdump)
                for m in range(MT):
                    gb = dbgp.tile([128, 2, 512], BF16, tag="gb", name=f"{R}gb{m}")
                    nc.scalar.dma_start(
                        out=gb,
                        in_=ag_in[m, :, :].rearrange("(c p) f -> p c f", p=128),
                    )
                    gf = dbgp.tile([128, 2, 512], F32, tag="gf", bufs=1, name=f"{R}gf{m}")
                    nc.vector.tensor_copy(gf, gb)
                    nc.scalar.dma_start(
                        out=d["agin"][m, :, :].rearrange("(c p) f -> p c f", p=128),
                        in_=gf,
                    )
                    for cc in range(4):
                        g2b = dbgp.tile(
                            [128, 2, 512], BF16, tag="gb", name=f"{R}g2b{m}_{cc}"
                        )
                        nc.scalar.dma_start(
                            out=g2b,
                            in_=ag_out[m, cc * PC : (cc + 1) * PC, :].rearrange(
                                "(c p) f -> p c f", p=128
                            ),
                        )
                        g2f = dbgp.tile(
                            [128, 2, 512], F32, tag="gf", bufs=1,
                            name=f"{R}g2f{m}_{cc}",
                        )
                        nc.vector.tensor_copy(g2f, g2b)
                        nc.scalar.dma_start(
                            out=d["agout"][m, cc * PC : (cc + 1) * PC, :].rearrange(
                                "(c p) f -> p c f", p=128
                            ),
                            in_=g2f,
                        )


def _make_in_maps(queries, keys, values, Wq, bq, Wk, bk, Wv, bv, Wo, bo):
    # bv folds through attention (softmax weights sum to 1) and the output
    # projection into an effective output bias; bk shifts every logit in a
    # row equally so softmax cancels it.
    bo_eff = bo + Wo @ bv
    c = np.ascontiguousarray
    in_maps = []
    for core in range(NCORES):
        b, r = core // 4, core % 4
        sl = slice(r * PC, (r + 1) * PC)
        in_maps.append(
            {
                "xq": c(queries[b]),
                "xk": c(keys[b]),
                "xv": c(values[b]),
                "wqT": c(Wq[sl, :].T),
                "wkT": c(Wk[sl, :].T),
                "wvT": c(Wv[sl, :].T),
                "woT": c(Wo.T[:, sl]),
                "bq": c(bq[sl].reshape(PC, 1)),
                "bo": c(bo_eff[sl].reshape(PC, 1)),
            }
        )
    return in_maps


def kernel(queries, keys, values, Wq, bq, Wk, bk, Wv, bv, Wo, bo, _trace=False):
    import concourse.bass_utils as bass_utils

    args = [queries, keys, values, Wq, bq, Wk, bk, Wv, bv, Wo, bo]
    args = [np.asarray(a, dtype=np.float32) for a in args]

    if "nc" not in _CACHE:
        _CACHE["nc"] = _build()
    nc = _CACHE["nc"]

    in_maps = _make_in_maps(*args)
    res = bass_utils.run_bass_kernel_spmd(
        nc, in_maps, core_ids=list(range(NCORES)), trace=_trace
    )
    _CACHE["last_result"] = res

    out = np.empty((B, M, D), dtype=np.float32)
    for core in range(NCORES):
        b, r = core // 4, core % 4
        out[b, :, r * PC : (r + 1) * PC] = res.results[core]["outT"].T
    return out


# revision 75
# speedup vs baseline: 1.4156x; 1.4156x over previous
"""Multi-head attention (B=2, M=N=2048, D=1024, H=16, DH=64) on 8 TRN2 cores.

Sharding: data-parallel over batch (cores 0-3 = batch 0, 4-7 = batch 1),
tensor-parallel over heads within each batch group (4 heads/core).

Engine plan (per core):
  PE      : x transposes (bf16-identity moving tensor => 1 cyc/row, data
            stays exact f32r), Q/K/V projections (f32r, F>=256), S^T
            matmuls (f32r, F=512), AV in O-natural orientation (bf16
            at/v2, F=65 incl. ones column for denominators), O^T
            transposes, out-projection (bf16).
  Act     : exp ONLY (the S^T->exp->AV chain is Act-limited; everything
            else is moved off this engine).
  DVE     : PSUM->SBUF copies, biases, reciprocal + per-query normalize.
  Pool    : half the stage-1 copies + weight DMAs.
  DMA     : chunked x loads, per-m bf16 AllGather feed, out stores.

Emission order = per-engine execution order (in-order engines), so the
program interleaves: K/Q chunk transposes+projections feed the first
S^T quanta at ~10us; a fill queue (V work, AV quanta, normalize, O^T
transpose + AllGather, out-projection) pops between S^T quanta to keep
PE busy while Act drains exps. Each m-chunk's (256,512) O^T shard
AllGathers (bf16) immediately so the out-projection pipelines under
later chunks' attention.

AV orientation: O natural [q partitions, d free]: lhsT = exp(S^T) tile
(keys on partitions), rhs = [V | ones] so column DH accumulates the
softmax denominator; normalization is then a per-partition DVE
tensor_scalar multiply (no gpsimd broadcast needed).

Host-side prep identical to the baseline: per-core transposed/sliced
weights, bv folded into bo_eff = bo + Wo @ bv, bk dropped (softmax
cancels row-constant logit shifts). Output assembly concatenates
per-core (256, 2048) out^T slices.
"""

import os

import numpy as np

B, M, NSEQ, D = 2, 2048, 2048, 1024
H, DH = 16, 64
HC = 4                # heads per core
PC = HC * DH          # 256 projected channels per core
CT = D // 128         # 8 contraction tiles
NT = NSEQ // 128      # 16 n-tiles
MT = M // 512         # 4 m-chunks
NCORES = 8

AT_BUFS = 12          # in-flight exp(S^T) tiles (bf16, 2KB/partition each)
FILL_NS = 1150.0      # non-S^T PE ns to interleave per S^T quantum

_CACHE = {}


def _build(single_core=False, reps=1):
    import concourse.bass as bass
    import concourse.tile as tile
    from concourse import bacc, mybir
    from concourse.masks import make_identity

    F32 = mybir.dt.float32
    F32R = mybir.dt.float32r
    BF16 = mybir.dt.bfloat16
    AF = mybir.ActivationFunctionType

    nc = bacc.Bacc(
        "TRN2",
        target_bir_lowering=False,
        debug=False,
        num_devices=1 if single_core else 8,
    )

    xq_d = nc.dram_tensor("xq", [M, D], F32, kind="ExternalInput")
    xk_d = nc.dram_tensor("xk", [NSEQ, D], F32, kind="ExternalInput")
    xv_d = nc.dram_tensor("xv", [NSEQ, D], F32, kind="ExternalInput")
    wqT_d = nc.dram_tensor("wqT", [D, PC], F32, kind="ExternalInput")
    wkT_d = nc.dram_tensor("wkT", [D, PC], F32, kind="ExternalInput")
    wvT_d = nc.dram_tensor("wvT", [D, PC], F32, kind="ExternalInput")
    woT_d = nc.dram_tensor("woT", [D, PC], F32, kind="ExternalInput")
    bq_d = nc.dram_tensor("bq", [PC, 1], F32, kind="ExternalInput")
    bo_d = nc.dram_tensor("bo", [PC, 1], F32, kind="ExternalInput")
    outT_d = nc.dram_tensor("outT", [PC, M], F32, kind="ExternalOutput")
    debug = bool(int(os.environ.get("KERNEL_DEBUG", "0")))
    dbg = None
    if debug:
        dbg = dict(
            kT=nc.dram_tensor("kT_dbg", [128, 2, NSEQ], F32, kind="ExternalOutput"),
            qT=nc.dram_tensor("qT_dbg", [128, 2, M], F32, kind="ExternalOutput"),
            v2=nc.dram_tensor(
                "v2_dbg", [128, HC, NT, DH + 1], F32, kind="ExternalOutput"
            ),
            agin=nc.dram_tensor("agin_dbg", [MT, PC, 512], F32, kind="ExternalOutput"),
            at0=nc.dram_tensor("at0_dbg", [128, 1024], F32, kind="ExternalOutput"),
            osb=nc.dram_tensor(
                "osb_dbg", [MT, 128, 4, PC], F32, kind="ExternalOutput"
            ),
        )

    with tile.TileContext(nc) as tc:
        with (
            tc.tile_pool(name="singles", bufs=1) as singles,
            tc.tile_pool(name="dram", bufs=1, space="DRAM") as dram,
        ):
            ident = singles.tile([128, 128], BF16)
            make_identity(nc, ident)
            # PE matmuls cannot mix 32-bit and 16-bit operands, and f32r
            # matmul inputs must come from a rounding producer — so the x
            # transposes stay f32 (the PSUM->SBUF copy rounds to f32r)
            ident_r = singles.tile([128, 128], F32)
            make_identity(nc, ident_r)
            bq_sb = singles.tile([128, 2], F32)
            nc.sync.dma_start(
                out=bq_sb, in_=bq_d[:, :].rearrange("(o p) w -> p (o w)", p=128)
            )
            bo_sb = singles.tile([128, 2], F32)
            nc.sync.dma_start(
                out=bo_sb, in_=bo_d[:, :].rearrange("(o p) w -> p (o w)", p=128)
            )

            ag_in = dram.tile([MT, PC, 512], BF16)
            # split gather: half h covers the two heads of pair h (channel
            # rows h*128..h*128+127 of each core's O^T shard), gathered
            # across the 4-core group as [cc, 128, 512]
            ag_out = dram.tile([MT, 2, 4, 128, 512], BF16)

            for rep in range(reps):
                _emit_rep(
                    nc, tc, bass, mybir, F32, F32R, BF16, AF, rep, single_core,
                    dict(
                        xq_d=xq_d, xk_d=xk_d, xv_d=xv_d, wqT_d=wqT_d,
                        wkT_d=wkT_d, wvT_d=wvT_d, woT_d=woT_d, outT_d=outT_d,
                        ident=ident, ident_r=ident_r, bq_sb=bq_sb,
                        bo_sb=bo_sb, ag_in=ag_in, ag_out=ag_out,
                        dbg=dbg if rep == reps - 1 else None,
                    ),
                )
    nc.compile()
    return nc


def _emit_rep(nc, tc, bass, mybir, F32, F32R, BF16, AF, rep, single_core, env):
    ident = env["ident"]
    ident_r = env["ident_r"]
    bq_sb, bo_sb = env["bq_sb"], env["bo_sb"]
    at_bufs_n = 8 if os.environ.get("KERNEL_DEBUG") == "1" else AT_BUFS
    ag_in, ag_out = env["ag_in"], env["ag_out"]
    R = f"r{rep}_"

    with (
        tc.tile_pool(name=f"{R}w", bufs=1) as w_pool,
        tc.tile_pool(name=f"{R}nat", bufs=9) as nat_pool,
        tc.tile_pool(name=f"{R}xT", bufs=2) as xT_pool,
        tc.tile_pool(name=f"{R}proj", bufs=1) as proj_pool,
        tc.tile_pool(name=f"{R}at", bufs=at_bufs_n) as at_pool,
        tc.tile_pool(
            name=f"{R}osb",
            bufs=1 if os.environ.get("KERNEL_DEBUG") == "1" else 2,
        ) as osb_pool,
        tc.tile_pool(
            name=f"{R}og",
            bufs=1 if os.environ.get("KERNEL_DEBUG") == "1" else 2,
        ) as og_pool,
        tc.tile_pool(name=f"{R}o", bufs=2) as o_pool,
        tc.tile_pool(name=f"{R}rec", bufs=4) as rec_pool,
        tc.tile_pool(name=f"{R}ps1", bufs=2, space="PSUM") as ps1,
        tc.tile_pool(name=f"{R}pss", bufs=2, space="PSUM") as ps_s_pool,
        tc.tile_pool(name=f"{R}pso", bufs=2, space="PSUM") as ps_o_pool,
    ):
        # ---- persistent per-rep tiles ----
        kT = proj_pool.tile([128, 2, NSEQ], F32R, name=f"{R}kT")
        qT = proj_pool.tile([128, 2, M], F32R, name=f"{R}qT")
        # AV rhs: [k(128), h, nt, 0:64]=V bf16, [..., 64]=ones (denominator)
        v2 = proj_pool.tile([128, HC, NT, DH + 1], BF16, name=f"{R}v2")
        nc.vector.memset(v2[:, :, :, DH : DH + 1], 1.0)

        xd = {"k": env["xk_d"], "q": env["xq_d"], "v": env["xv_d"]}

        # ---- emit helpers ----
        def psum_copy(dst, src):
            # all PSUM->SBUF copies on DVE: GPSIMD has no PSUM access and
            # Act is reserved for the exp stream
            nc.vector.tensor_copy(dst, src)

        # x chunk loads: 4 per-row-block DMAs per (tensor, chunk), issued in
        # expected consumption order, ~2 chunks ahead of the transposes
        LOADS = [("q", 0), ("k", 0), ("k", 1), ("v", 0), ("k", 2), ("v", 1),
                 ("k", 3), ("v", 2), ("v", 3), ("q", 1), ("q", 2), ("q", 3)]
        load_idx = [0]
        nat = {}

        def load_chunk(t, c):
            if (t, c) in nat:
                return
            tiles = []
            for i in range(4):
                nt_t = nat_pool.tile(
                    [128, D], F32, tag="nat", name=f"{R}nat_{t}{c}_{i}"
                )
                nc.sync.dma_start(
                    out=nt_t,
                    in_=xd[t][c * 512 + i * 128 : c * 512 + (i + 1) * 128, :],
                )
                tiles.append(nt_t)
            nat[(t, c)] = tiles

        def load_next():
            while load_idx[0] < len(LOADS) and LOADS[load_idx[0]] in nat:
                load_idx[0] += 1
            if load_idx[0] < len(LOADS):
                load_chunk(*LOADS[load_idx[0]])
                load_idx[0] += 1

        xT = {}

        def tr_tile(t, c, i):
            # transpose one 128-row tile of chunk c across all 8 ct blocks;
            # per-tile granularity lets the PE start as soon as one DMA
            # lands instead of waiting for the whole 512-row chunk
            if i == 0:
                load_chunk(t, c)  # JIT fallback; normally already loaded
                load_next()       # keep prefetch lead
                x_t = xT_pool.tile(
                    [128, CT, 512], F32R, tag="xT", name=f"{R}xT_{t}{c}"
                )
                xT[(t, c)] = x_t
            x_t = xT[(t, c)]
            tile_i = nat[(t, c)][i]
            for ch in range(2):
                pst = ps1.tile(
                    [128, 512], F32, tag="ps1", name=f"{R}pst{t}{c}_{i}_{ch}"
                )
                for k in range(4):
                    ct = ch * 4 + k
                    nc.tensor.transpose(
                        pst[:, k * 128 : (k + 1) * 128],
                        tile_i[:, ct * 128 : (ct + 1) * 128],
                        ident_r,
                    )
                psum_copy(
                    x_t[:, ch * 4 : (ch + 1) * 4, i * 128 : (i + 1) * 128],
                    pst[:, :].rearrange("p (k f) -> p k f", k=4),
                )

        def tr_chunk(t, c, half):
            for i in (2 * half, 2 * half + 1):
                tr_tile(t, c, i)

        def kq_proj(t, c, ot):
            dst_T = kT if t == "k" else qT
            pj = ps1.tile([128, 512], F32, tag="ps1", name=f"{R}pj{t}{c}_{ot}")
            for ct in range(CT):
                nc.tensor.matmul(
                    pj,
                    w_sb[t][:, ct, ot * 128 : (ot + 1) * 128],
                    xT[(t, c)][:, ct, :],
                    start=(ct == 0),
                    stop=(ct == CT - 1),
                )
            dst = dst_T[:, ot, c * 512 : (c + 1) * 512]
            if t == "q":
                nc.vector.tensor_scalar_add(dst, pj, bq_sb[:, ot : ot + 1])
            else:
                psum_copy(dst, pj)

        vproj_done = [0]

        def v_proj(ntile):
            c = ntile // 4
            psv = ps1.tile([128, PC], F32, tag="ps1", name=f"{R}psv{ntile}")
            for ct in range(CT):
                nc.tensor.matmul(
                    psv,
                    xT[("v", c)][:, ct, ntile % 4 * 128 : (ntile % 4 + 1) * 128],
                    w_sb["v"][:, ct, :],
                    start=(ct == 0),
                    stop=(ct == CT - 1),
                )
            nc.vector.tensor_copy(
                v2[:, :, ntile, 0:DH],
                psv[:, :].rearrange("p (h d) -> p h d", h=HC),
            )
            vproj_done[0] = ntile + 1

        at_tiles = {}
        pso = {}

        def st_quantum(m, p, ntile):
            ps_s = ps_s_pool.tile(
                [128, 1024], F32, tag="pss", name=f"{R}pss{m}_{p}_{ntile}"
            )
            for j in range(2):
                base = j * 64
                nc.tensor.matmul(
                    ps_s[:, j * 512 : (j + 1) * 512],
                    kT[base : base + 64, p, ntile * 128 : (ntile + 1) * 128],
                    qT[base : base + 64, p, m * 512 : (m + 1) * 512],
                    start=True,
                    stop=True,
                )
            at = at_pool.tile(
                [128, 1024], BF16, tag="at", name=f"{R}at{m}_{p}_{ntile}"
            )
            nc.scalar.activation(at, ps_s, AF.Exp)
            at_tiles[(m, p, ntile)] = at
            if env.get("dbg") and (m, p, ntile) == (0, 0, 0):
                atf = proj_pool.tile([128, 1024], F32, name=f"{R}atf")
                nc.vector.tensor_copy(atf, at)
                nc.sync.dma_start(out=env["dbg"]["at0"][:, :], in_=atf)

        def av_quantum(m, p, ntile):
            if ntile == 0:
                for j in range(2):
                    # [128, 4, 128] = exactly one 2KB PSUM bank per
                    # accumulator — avoids two accumulation groups sharing
                    # a bank (start=True behavior is then self-contained)
                    pso[(m, p, j)] = ps_o_pool.tile(
                        [128, 4, 128], F32, tag="pso",
                        name=f"{R}pso{m}_{p}_{j}",
                    )
            at = at_tiles.pop((m, p, ntile))
            for j in range(2):
                for qt in range(4):
                    # start=True zeroes the WHOLE 2KB PSUM bank, so only
                    # the very first matmul of the bank's accumulation
                    # (qt==0, nt==0) may set it; the other qt regions then
                    # accumulate from the zeroed state
                    nc.tensor.matmul(
                        pso[(m, p, j)][:, qt, 0 : DH + 1],
                        at[:, j * 512 + qt * 128 : j * 512 + (qt + 1) * 128],
                        v2[:, 2 * p + j, ntile, :],
                        start=(ntile == 0 and qt == 0),
                        stop=(ntile == NT - 1 and qt == 3),
                        skip_group_check=True,
                    )

        o_sb = {}

        def norm(m, p):
            if p == 0:
                o_sb[m] = o_pool.tile([128, 4, PC], F32, tag="o", name=f"{R}o{m}")
            for j in range(2):
                h = 2 * p + j
                ps = pso.pop((m, p, j))
                rec = rec_pool.tile(
                    [128, 4, 1], F32, tag="rec", name=f"{R}rec{m}_{h}"
                )
                nc.vector.reciprocal(rec, ps[:, :, DH : DH + 1])
                for qt in range(4):
                    nc.vector.tensor_scalar_mul(
                        o_sb[m][:, qt, h * DH : (h + 1) * DH],
                        ps[:, qt, 0:DH],
                        rec[:, qt, 0:1],
                    )

        def otr_ag(m, half):
            # transpose + gather ONE head-pair's 128-channel half of O^T:
            # half 0 (heads 2p=0,1, channels 0..127) launches right after
            # norm(m, p=0), ~half an m-block before half 1 — so the
            # AllGather latency hides under the remaining S^T stream.
            # f32 transposes (PSUM banks are f32-native); the SBUF copy
            # casts to bf16 for the AllGather.
            otr = ps_s_pool.tile(
                [128, 512], F32, tag="pss", name=f"{R}otr{m}_{half}"
            )
            for qt in range(4):
                nc.tensor.transpose(
                    otr[:, qt * 128 : (qt + 1) * 128],
                    o_sb[m][:, qt, half * 128 : (half + 1) * 128],
                    ident_r,
                )
            otr_sb = o_pool.tile(
                [128, 512], BF16, tag="otrsb", bufs=2, name=f"{R}otrsb{m}_{half}"
            )
            nc.vector.tensor_copy(otr_sb, otr)
            if env.get("dbg") and half == 1:
                nc.sync.dma_start(
                    out=env["dbg"]["osb"][m, :, :, :], in_=o_sb[m]
                )
            # scalar-engine DMA queue: keeps the gather path ordered among
            # itself but independent of the (long) nat-load sync queue
            nc.scalar.dma_start(
                out=ag_in[m, half * 128 : (half + 1) * 128, :].rearrange(
                    "(c p) f -> p c f", p=128
                ),
                in_=otr_sb,
            )
            if single_core:
                for rr in range(4):
                    nc.scalar.dma_start(
                        out=ag_out[m, half, rr, :, :],
                        in_=ag_in[m, half * 128 : (half + 1) * 128, :],
                    )
            else:
                nc.gpsimd.collective_compute(
                    "AllGather",
                    bass.mybir.AluOpType.bypass,
                    replica_groups=[[0, 1, 2, 3], [4, 5, 6, 7]],
                    ins=[ag_in[m, half * 128 : (half + 1) * 128, :].opt()],
                    outs=[ag_out[m, half, :, :, :].opt()],
                )

        og = {}
        osb = {}

        def og_load(m):
            # og ct blocks: gathered channel block ct = cc*2 + half
            # (core cc's heads, pair half) -> og[:, ct, :]
            og[m] = og_pool.tile([128, CT, 512], BF16, tag="og", name=f"{R}og{m}")
            for half in range(2):
                nc.scalar.dma_start(
                    out=og[m][:, :, :].rearrange(
                        "p (cc hh) f -> p hh cc f", hh=2
                    )[:, half, :, :],
                    in_=ag_out[m, half, :, :, :].rearrange(
                        "cc p f -> p cc f"
                    ),
                )
            osb[m] = osb_pool.tile([128, 2, 512], F32, tag="osb", name=f"{R}osb{m}")

        def out_proj(m, ot):
            po = ps_s_pool.tile([128, 512], F32, tag="pss", name=f"{R}po{m}_{ot}")
            for ct in range(CT):
                nc.tensor.matmul(
                    po,
                    wo_bf[:, ct, ot * 128 : (ot + 1) * 128],
                    og[m][:, ct, :],
                    start=(ct == 0),
                    stop=(ct == CT - 1),
                )
            nc.vector.tensor_scalar_add(
                osb[m][:, ot, :], po, bo_sb[:, ot : ot + 1]
            )
            if ot == 1:
                nc.scalar.dma_start(
                    out=env["outT_d"][:, m * 512 : (m + 1) * 512].rearrange(
                        "(o p) f -> p o f", p=128
                    ),
                    in_=osb[m],
                )
                og.pop(m)
                osb.pop(m)

        # ---- fill-queue scheduler ----
        emit_log = []
        log_on = bool(int(os.environ.get("KERNEL_EMIT_LOG", "0")))
        fill = []
        fill_hi = []  # popped first: Q chains (unblock the next m's S^T)
        deferred = []  # (ready_slot, item): held until slot counter passes
        slot = [0]
        deficit = [0.0]

        def release_deferred():
            while deferred and deferred[0][0] <= slot[0]:
                fill.append(deferred.pop(0)[1])

        def pop_one():
            item = (fill_hi or fill).pop(0)
            if len(item) == 3:
                ns, fn, label = item
            else:
                (ns, fn), label = item, getattr(item[1], "__name__", "?")
            if log_on:
                emit_log.append(f"pop:{label}")
            fn()
            return ns

        def pop_fill(budget):
            deficit[0] = min(deficit[0] + budget, 4 * FILL_NS)
            while (fill or fill_hi) and deficit[0] > 0:
                deficit[0] -= pop_one()

        kq_emitted = set()

        def ensure_kq(t, c):
            if (t, c) in kq_emitted:
                return
            kq_emitted.add((t, c))
            tr_chunk(t, c, 0)
            tr_chunk(t, c, 1)
            kq_proj(t, c, 0)
            kq_proj(t, c, 1)

        # prefetch the first two chunks (q0 for Qproj(m0), k0 for S^T nt0)
        # BEFORE the weight DMAs — the first transposes are the critical
        # path; weights aren't read until the first projection (~7us in)
        load_next()
        load_next()

        w_sb = {}
        for tname, wd in (("k", "wkT_d"), ("q", "wqT_d"), ("v", "wvT_d")):
            w_sb[tname] = w_pool.tile([128, CT, PC], F32R, name=f"{R}w{tname}")
            nc.gpsimd.dma_start(
                out=w_sb[tname],
                in_=env[wd][:, :].rearrange("(ct p) c -> p ct c", p=128),
            )
        wo_f32 = nat_pool.tile(
            [128, CT, PC], F32, tag="wof", bufs=1, name=f"{R}wof"
        )
        nc.gpsimd.dma_start(
            out=wo_f32,
            in_=env["woT_d"][:, :].rearrange("(ct p) c -> p ct c", p=128),
        )
        wo_bf = w_pool.tile([128, CT, PC], BF16, name=f"{R}wob")
        nc.vector.tensor_copy(wo_bf, wo_f32)

        v_enqueued = [False]

        def enqueue_v_chain():
            if v_enqueued[0]:
                return
            v_enqueued[0] = True
            for c in range(4):
                fill.append(
                    (900.0, lambda c=c: tr_chunk("v", c, 0), f"Vtr0:{c}")
                )
                fill.append(
                    (900.0, lambda c=c: tr_chunk("v", c, 1), f"Vtr1:{c}")
                )
                for k in range(4):
                    fill.append(
                        (900.0, lambda n=c * 4 + k: v_proj(n),
                         f"Vproj:{c * 4 + k}")
                    )

        av_pending = []

        def flush_av():
            while av_pending and vproj_done[0] > av_pending[0][2]:
                mm, pp, ntile = av_pending.pop(0)
                fill.append(
                    (450.0, lambda a=mm, b=pp, c=ntile: av_quantum(a, b, c),
                     f"AV:{mm}{pp}.{ntile}")
                )

        # ---- main S^T driver ----
        for m in range(MT):
            # finish any in-flight pre-emitted Q chain for this m first
            while m > 0 and ("q", m) not in kq_emitted and (fill or fill_hi):
                flush_av()
                pop_one()
            ensure_kq("q", m)
            for p in range(2):
                for ntile in range(NT):
                    ensure_kq("k", ntile // 4)
                    if m == 0 and p == 0 and ntile == 4:
                        enqueue_v_chain()
                    if m < MT - 1 and p == 1 and ntile == 4:
                        # pre-emit next m's Q chain (high priority) so the
                        # m boundary doesn't stall the S^T/exp stream
                        nm = m + 1
                        fill_hi.append(
                            (900.0, lambda c=nm: tr_chunk("q", c, 0),
                             f"Qtr0:{nm}")
                        )
                        fill_hi.append(
                            (900.0, lambda c=nm: tr_chunk("q", c, 1),
                             f"Qtr1:{nm}")
                        )
                        fill_hi.append(
                            (900.0, lambda c=nm: kq_proj("q", c, 0),
                             f"Qproj0:{nm}")
                        )

                        def q_done(c=nm):
                            kq_proj("q", c, 1)
                            kq_emitted.add(("q", c))

                        fill_hi.append((900.0, q_done, f"Qproj1:{nm}"))
                    # at-pool headroom: force-drain fill before exp would
                    # block the Act engine on a full at pool (deadlock via
                    # the in-order PE stream otherwise)
                    while len(at_tiles) >= at_bufs_n - 1 and (fill or fill_hi):
                        flush_av()
                        pop_one()
                    if log_on:
                        emit_log.append(f"ST:{m}{p}.{ntile}")
                    st_quantum(m, p, ntile)
                    slot[0] += 1
                    release_deferred()
                    av_pending.append((m, p, ntile))
                    flush_av()
                    pop_fill(FILL_NS)

                def block_tail(m=m, p=p):
                    norm(m, p)
                    otr_ag(m, p)
                    if p == 1:
                        og_load(m)
                        # hold the out-projection back ~20 S^T slots so the
                        # in-order PE stream doesn't hit it before the
                        # AllGather + og DMA have landed
                        deferred.append(
                            (slot[0] + 20,
                             (1750.0, lambda: out_proj(m, 0), f"oproj0:{m}"))
                        )
                        deferred.append(
                            (slot[0] + 20,
                             (1750.0, lambda: out_proj(m, 1), f"oproj1:{m}"))
                        )
                fill.append((500.0, block_tail, f"tail:{m}{p}"))

        # drain
        while av_pending or fill or fill_hi or deferred:
            slot[0] += 10_000
            release_deferred()
            flush_av()
            if fill or fill_hi:
                pop_one()
            else:
                assert not av_pending, "AV stuck without V projections"
        if log_on:
            print("EMIT ORDER:", " ".join(emit_log))

        if env.get("dbg"):
            d = env["dbg"]
            nc.sync.dma_start(out=d["kT"][:, :, :], in_=kT.bitcast(F32))
            nc.sync.dma_start(out=d["qT"][:, :, :], in_=qT.bitcast(F32))
            with tc.tile_pool(name=f"{R}dbgp", bufs=2) as dbgp:
                for h in range(HC):
                    vdump = dbgp.tile(
                        [128, NT, DH + 1], F32, tag="vd", bufs=1,
                        name=f"{R}vdump{h}",
                    )
                    nc.vector.tensor_copy(vdump, v2[:, h, :, :])
                    nc.sync.dma_start(out=d["v2"][:, h, :, :], in_=vdump)
                for m in range(MT):
                    gb = dbgp.tile([128, 2, 512], BF16, tag="gb", name=f"{R}gb{m}")
                    nc.scalar.dma_start(
                        out=gb,
                        in_=ag_in[m, :, :].rearrange("(c p) f -> p c f", p=128),
                    )
                    gf = dbgp.tile([128, 2, 512], F32, tag="gf", bufs=1, name=f"{R}gf{m}")
                    nc.vector.tensor_copy(gf, gb)
                    nc.scalar.dma_start(
                        out=d["agin"][m, :, :].rearrange("(c p) f -> p c f", p=128),
                        in_=gf,
                    )



def _make_in_maps(queries, keys, values, Wq, bq, Wk, bk, Wv, bv, Wo, bo):
    # bv folds through attention (softmax weights sum to 1) and the output
    # projection into an effective output bias; bk shifts every logit in a
    # row equally so softmax cancels it.
    bo_eff = bo + Wo @ bv
    c = np.ascontiguousarray
    in_maps = []
    for core in range(NCORES):
        b, r = core // 4, core % 4
        sl = slice(r * PC, (r + 1) * PC)
        in_maps.append(
            {
                "xq": c(queries[b]),
                "xk": c(keys[b]),
                "xv": c(values[b]),
                "wqT": c(Wq[sl, :].T),
                "wkT": c(Wk[sl, :].T),
                "wvT": c(Wv[sl, :].T),
                "woT": c(Wo.T[:, sl]),
                "bq": c(bq[sl].reshape(PC, 1)),
                "bo": c(bo_eff[sl].reshape(PC, 1)),
            }
        )
    return in_maps


def kernel(queries, keys, values, Wq, bq, Wk, bk, Wv, bv, Wo, bo, _trace=False):
    import concourse.bass_utils as bass_utils

    args = [queries, keys, values, Wq, bq, Wk, bk, Wv, bv, Wo, bo]
    args = [np.asarray(a, dtype=np.float32) for a in args]

    if "nc" not in _CACHE:
        _CACHE["nc"] = _build()
    nc = _CACHE["nc"]

    in_maps = _make_in_maps(*args)
    res = bass_utils.run_bass_kernel_spmd(
        nc, in_maps, core_ids=list(range(NCORES)), trace=_trace
    )
    _CACHE["last_result"] = res

    out = np.empty((B, M, D), dtype=np.float32)
    for core in range(NCORES):
        b, r = core // 4, core % 4
        out[b, :, r * PC : (r + 1) * PC] = res.results[core]["outT"].T
    return out


# revision 76
# speedup vs baseline: 1.8011x; 1.2723x over previous
"""Multi-head attention (B=2, M=N=2048, D=1024, H=16, DH=64) on 8 TRN2 cores.

Sharding: data-parallel over batch (cores 0-3 = batch 0, 4-7 = batch 1),
tensor-parallel over heads within each batch group (4 heads/core).

Engine plan (per core):
  PE      : x transposes (f32, 2 cyc/row — the walrus verifier forbids
            mixed 32/16-bit matmul operands and non-rounded f32r inputs),
            Q/K/V projections (f32r, F>=256 => 1 cyc/row), S^T matmuls
            (f32r, F=512), AV in O-natural orientation (bf16 at/v2, F=65
            incl. a ones column accumulating softmax denominators), O^T
            transposes (f32), out-projection (bf16).
  Act     : exp ONLY (the S^T->exp->AV chain is Act-heavy; everything
            else is moved off this engine).
  DVE     : all PSUM->SBUF copies (GPSIMD cannot access PSUM), biases,
            reciprocal + per-query normalize (tensor_scalar, since the
            O-natural layout puts denominators on the partition axis —
            no gpsimd broadcast needed).
  Pool    : weight DMAs, memsets.
  DMA     : per-128-row x tile loads (sync queue), bf16 gather path
            (scalar queue), out stores.

Emission order = per-engine execution order (in-order engines). A
fill-queue scheduler interleaves everything around the S^T stream: K/Q
chunk transposes+projections feed the first S^T quanta at ~13us; V
work, AV quanta, normalize, O^T-transpose+AllGather and the (deferred)
out-projection pop between S^T quanta so PE stays busy while Act
drains exps. Each m-chunk's O^T shard AllGathers per 128-channel
head-pair half (the p0 half launches ~half a block early) so the
gather latency hides under the remaining S^T stream.

PSUM note: matmul start=True zeroes the ENTIRE 2KB bank, so the four
per-qt AV accumulation regions sharing one bank set start only on the
bank's very first matmul (qt==0, nt==0) and accumulate from the zeroed
state otherwise (skip_group_check).

Host-side prep identical to the baseline: per-core transposed/sliced
weights, bv folded into bo_eff = bo + Wo @ bv, bk dropped (softmax
cancels row-constant logit shifts). Output assembly concatenates
per-core (256, 2048) out^T slices.
"""

import os

import numpy as np

B, M, NSEQ, D = 2, 2048, 2048, 1024
H, DH = 16, 64
HC = 4                # heads per core
PC = HC * DH          # 256 projected channels per core
CT = D // 128         # 8 contraction tiles
NT = NSEQ // 128      # 16 n-tiles
MT = M // 512         # 4 m-chunks
NCORES = 8

AT_BUFS = 12          # in-flight exp(S^T) tiles (bf16, 2KB/partition each)
FILL_NS = 1150.0      # non-S^T PE ns to interleave per S^T quantum

_CACHE = {}


def _build(single_core=False, reps=1):
    import concourse.bass as bass
    import concourse.tile as tile
    from concourse import bacc, mybir
    from concourse.masks import make_identity

    F32 = mybir.dt.float32
    F32R = mybir.dt.float32r
    BF16 = mybir.dt.bfloat16
    AF = mybir.ActivationFunctionType

    nc = bacc.Bacc(
        "TRN2",
        target_bir_lowering=False,
        debug=False,
        num_devices=1 if single_core else 8,
    )

    xq_d = nc.dram_tensor("xq", [M, D], F32, kind="ExternalInput")
    xk_d = nc.dram_tensor("xk", [NSEQ, D], F32, kind="ExternalInput")
    xv_d = nc.dram_tensor("xv", [NSEQ, D], F32, kind="ExternalInput")
    wqT_d = nc.dram_tensor("wqT", [D, PC], F32, kind="ExternalInput")
    wkT_d = nc.dram_tensor("wkT", [D, PC], F32, kind="ExternalInput")
    wvT_d = nc.dram_tensor("wvT", [D, PC], F32, kind="ExternalInput")
    woT_d = nc.dram_tensor("woT", [D, PC], F32, kind="ExternalInput")
    bq_d = nc.dram_tensor("bq", [PC, 1], F32, kind="ExternalInput")
    bo_d = nc.dram_tensor("bo", [PC, 1], F32, kind="ExternalInput")
    outT_d = nc.dram_tensor("outT", [PC, M], F32, kind="ExternalOutput")
    debug = bool(int(os.environ.get("KERNEL_DEBUG", "0")))
    dbg = None
    if debug:
        dbg = dict(
            kT=nc.dram_tensor("kT_dbg", [128, 2, NSEQ], F32, kind="ExternalOutput"),
            qT=nc.dram_tensor("qT_dbg", [128, 2, M], F32, kind="ExternalOutput"),
            v2=nc.dram_tensor(
                "v2_dbg", [128, HC, NT, DH + 1], F32, kind="ExternalOutput"
            ),
            agin=nc.dram_tensor("agin_dbg", [MT, PC, 512], F32, kind="ExternalOutput"),
            at0=nc.dram_tensor("at0_dbg", [128, 1024], F32, kind="ExternalOutput"),
            osb=nc.dram_tensor(
                "osb_dbg", [MT, 128, 4, PC], F32, kind="ExternalOutput"
            ),
        )

    with tile.TileContext(nc) as tc:
        with (
            tc.tile_pool(name="singles", bufs=1) as singles,
            tc.tile_pool(name="dram", bufs=1, space="DRAM") as dram,
        ):
            ident = singles.tile([128, 128], BF16)
            make_identity(nc, ident)
            # PE matmuls cannot mix 32-bit and 16-bit operands, and f32r
            # matmul inputs must come from a rounding producer — so the x
            # transposes stay f32 (the PSUM->SBUF copy rounds to f32r)
            ident_r = singles.tile([128, 128], F32)
            make_identity(nc, ident_r)
            bq_sb = singles.tile([128, 2], F32)
            nc.sync.dma_start(
                out=bq_sb, in_=bq_d[:, :].rearrange("(o p) w -> p (o w)", p=128)
            )
            bo_sb = singles.tile([128, 2], F32)
            nc.sync.dma_start(
                out=bo_sb, in_=bo_d[:, :].rearrange("(o p) w -> p (o w)", p=128)
            )

            ag_in = dram.tile([MT, PC, 512], BF16)
            # split gather: half h covers the two heads of pair h (channel
            # rows h*128..h*128+127 of each core's O^T shard), gathered
            # across the 4-core group as [cc, 128, 512]
            ag_out = dram.tile([MT, 2, 4, 128, 512], BF16)

            for rep in range(reps):
                _emit_rep(
                    nc, tc, bass, mybir, F32, F32R, BF16, AF, rep, single_core,
                    dict(
                        xq_d=xq_d, xk_d=xk_d, xv_d=xv_d, wqT_d=wqT_d,
                        wkT_d=wkT_d, wvT_d=wvT_d, woT_d=woT_d, outT_d=outT_d,
                        ident=ident, ident_r=ident_r, bq_sb=bq_sb,
                        bo_sb=bo_sb, ag_in=ag_in, ag_out=ag_out,
                        dbg=dbg if rep == reps - 1 else None,
                    ),
                )
    nc.compile()
    return nc


def _emit_rep(nc, tc, bass, mybir, F32, F32R, BF16, AF, rep, single_core, env):
    ident = env["ident"]
    ident_r = env["ident_r"]
    bq_sb, bo_sb = env["bq_sb"], env["bo_sb"]
    at_bufs_n = 8 if os.environ.get("KERNEL_DEBUG") == "1" else AT_BUFS
    ag_in, ag_out = env["ag_in"], env["ag_out"]
    R = f"r{rep}_"

    with (
        tc.tile_pool(name=f"{R}w", bufs=1) as w_pool,
        tc.tile_pool(name=f"{R}nat", bufs=9) as nat_pool,
        tc.tile_pool(name=f"{R}xT", bufs=2) as xT_pool,
        tc.tile_pool(name=f"{R}proj", bufs=1) as proj_pool,
        tc.tile_pool(name=f"{R}at", bufs=at_bufs_n) as at_pool,
        tc.tile_pool(
            name=f"{R}osb",
            bufs=1 if os.environ.get("KERNEL_DEBUG") == "1" else 2,
        ) as osb_pool,
        tc.tile_pool(
            name=f"{R}og",
            bufs=1 if os.environ.get("KERNEL_DEBUG") == "1" else 2,
        ) as og_pool,
        tc.tile_pool(name=f"{R}o", bufs=2) as o_pool,
        tc.tile_pool(name=f"{R}rec", bufs=4) as rec_pool,
        tc.tile_pool(name=f"{R}ps1", bufs=2, space="PSUM") as ps1,
        tc.tile_pool(name=f"{R}pss", bufs=2, space="PSUM") as ps_s_pool,
        tc.tile_pool(name=f"{R}pso", bufs=2, space="PSUM") as ps_o_pool,
    ):
        # ---- persistent per-rep tiles ----
        kT = proj_pool.tile([128, 2, NSEQ], F32R, name=f"{R}kT")
        qT = proj_pool.tile([128, 2, M], F32R, name=f"{R}qT")
        # AV rhs: [k(128), h, nt, 0:64]=V bf16, [..., 64]=ones (denominator)
        v2 = proj_pool.tile([128, HC, NT, DH + 1], BF16, name=f"{R}v2")
        nc.vector.memset(v2[:, :, :, DH : DH + 1], 1.0)

        xd = {"k": env["xk_d"], "q": env["xq_d"], "v": env["xv_d"]}

        # ---- emit helpers ----
        def psum_copy(dst, src):
            # all PSUM->SBUF copies on DVE: GPSIMD has no PSUM access and
            # Act is reserved for the exp stream
            nc.vector.tensor_copy(dst, src)

        # x chunk loads: 4 per-row-block DMAs per (tensor, chunk), issued in
        # expected consumption order, ~2 chunks ahead of the transposes
        LOADS = [("q", 0), ("k", 0), ("k", 1), ("v", 0), ("k", 2), ("v", 1),
                 ("k", 3), ("v", 2), ("v", 3), ("q", 1), ("q", 2), ("q", 3)]
        load_idx = [0]
        nat = {}

        def load_chunk(t, c):
            if (t, c) in nat:
                return
            tiles = []
            for i in range(4):
                nt_t = nat_pool.tile(
                    [128, D], F32, tag="nat", name=f"{R}nat_{t}{c}_{i}"
                )
                nc.sync.dma_start(
                    out=nt_t,
                    in_=xd[t][c * 512 + i * 128 : c * 512 + (i + 1) * 128, :],
                )
                tiles.append(nt_t)
            nat[(t, c)] = tiles

        def load_next():
            while load_idx[0] < len(LOADS) and LOADS[load_idx[0]] in nat:
                load_idx[0] += 1
            if load_idx[0] < len(LOADS):
                load_chunk(*LOADS[load_idx[0]])
                load_idx[0] += 1

        xT = {}

        def tr_tile(t, c, i):
            # transpose one 128-row tile of chunk c across all 8 ct blocks;
            # per-tile granularity lets the PE start as soon as one DMA
            # lands instead of waiting for the whole 512-row chunk
            if i == 0:
                load_chunk(t, c)  # JIT fallback; normally already loaded
                load_next()       # keep prefetch lead
                x_t = xT_pool.tile(
                    [128, CT, 512], F32R, tag="xT", name=f"{R}xT_{t}{c}"
                )
                xT[(t, c)] = x_t
            x_t = xT[(t, c)]
            tile_i = nat[(t, c)][i]
            for ch in range(2):
                pst = ps1.tile(
                    [128, 512], F32, tag="ps1", name=f"{R}pst{t}{c}_{i}_{ch}"
                )
                for k in range(4):
                    ct = ch * 4 + k
                    nc.tensor.transpose(
                        pst[:, k * 128 : (k + 1) * 128],
                        tile_i[:, ct * 128 : (ct + 1) * 128],
                        ident_r,
                    )
                psum_copy(
                    x_t[:, ch * 4 : (ch + 1) * 4, i * 128 : (i + 1) * 128],
                    pst[:, :].rearrange("p (k f) -> p k f", k=4),
                )

        def tr_chunk(t, c, half):
            for i in (2 * half, 2 * half + 1):
                tr_tile(t, c, i)

        def kq_proj(t, c, ot):
            dst_T = kT if t == "k" else qT
            pj = ps1.tile([128, 512], F32, tag="ps1", name=f"{R}pj{t}{c}_{ot}")
            for ct in range(CT):
                nc.tensor.matmul(
                    pj,
                    w_sb[t][:, ct, ot * 128 : (ot + 1) * 128],
                    xT[(t, c)][:, ct, :],
                    start=(ct == 0),
                    stop=(ct == CT - 1),
                )
            dst = dst_T[:, ot, c * 512 : (c + 1) * 512]
            if t == "q":
                nc.vector.tensor_scalar_add(dst, pj, bq_sb[:, ot : ot + 1])
            else:
                psum_copy(dst, pj)

        vproj_done = [0]

        def v_proj(ntile):
            c = ntile // 4
            psv = ps1.tile([128, PC], F32, tag="ps1", name=f"{R}psv{ntile}")
            for ct in range(CT):
                nc.tensor.matmul(
                    psv,
                    xT[("v", c)][:, ct, ntile % 4 * 128 : (ntile % 4 + 1) * 128],
                    w_sb["v"][:, ct, :],
                    start=(ct == 0),
                    stop=(ct == CT - 1),
                )
            nc.vector.tensor_copy(
                v2[:, :, ntile, 0:DH],
                psv[:, :].rearrange("p (h d) -> p h d", h=HC),
            )
            vproj_done[0] = ntile + 1

        at_tiles = {}
        pso = {}

        def st_quantum(m, p, ntile):
            ps_s = ps_s_pool.tile(
                [128, 1024], F32, tag="pss", name=f"{R}pss{m}_{p}_{ntile}"
            )
            for j in range(2):
                base = j * 64
                nc.tensor.matmul(
                    ps_s[:, j * 512 : (j + 1) * 512],
                    kT[base : base + 64, p, ntile * 128 : (ntile + 1) * 128],
                    qT[base : base + 64, p, m * 512 : (m + 1) * 512],
                    start=True,
                    stop=True,
                )
            at = at_pool.tile(
                [128, 1024], BF16, tag="at", name=f"{R}at{m}_{p}_{ntile}"
            )
            nc.scalar.activation(at, ps_s, AF.Exp)
            at_tiles[(m, p, ntile)] = at
            if env.get("dbg") and (m, p, ntile) == (0, 0, 0):
                atf = proj_pool.tile([128, 1024], F32, name=f"{R}atf")
                nc.vector.tensor_copy(atf, at)
                nc.sync.dma_start(out=env["dbg"]["at0"][:, :], in_=atf)

        def av_quantum(m, p, ntile):
            if ntile == 0:
                for j in range(2):
                    # [128, 4, 128] = exactly one 2KB PSUM bank per
                    # accumulator — avoids two accumulation groups sharing
                    # a bank (start=True behavior is then self-contained)
                    pso[(m, p, j)] = ps_o_pool.tile(
                        [128, 4, 128], F32, tag="pso",
                        name=f"{R}pso{m}_{p}_{j}",
                    )
            at = at_tiles.pop((m, p, ntile))
            for j in range(2):
                for qt in range(4):
                    # start=True zeroes the WHOLE 2KB PSUM bank, so only
                    # the very first matmul of the bank's accumulation
                    # (qt==0, nt==0) may set it; the other qt regions then
                    # accumulate from the zeroed state
                    nc.tensor.matmul(
                        pso[(m, p, j)][:, qt, 0 : DH + 1],
                        at[:, j * 512 + qt * 128 : j * 512 + (qt + 1) * 128],
                        v2[:, 2 * p + j, ntile, :],
                        start=(ntile == 0 and qt == 0),
                        stop=(ntile == NT - 1 and qt == 3),
                        skip_group_check=True,
                    )

        o_sb = {}

        def norm(m, p):
            if p == 0:
                o_sb[m] = o_pool.tile([128, 4, PC], F32, tag="o", name=f"{R}o{m}")
            for j in range(2):
                h = 2 * p + j
                ps = pso.pop((m, p, j))
                rec = rec_pool.tile(
                    [128, 4, 1], F32, tag="rec", name=f"{R}rec{m}_{h}"
                )
                nc.vector.reciprocal(rec, ps[:, :, DH : DH + 1])
                for qt in range(4):
                    nc.vector.tensor_scalar_mul(
                        o_sb[m][:, qt, h * DH : (h + 1) * DH],
                        ps[:, qt, 0:DH],
                        rec[:, qt, 0:1],
                    )

        def otr_ag(m, half):
            # transpose + gather ONE head-pair's 128-channel half of O^T:
            # half 0 (heads 2p=0,1, channels 0..127) launches right after
            # norm(m, p=0), ~half an m-block before half 1 — so the
            # AllGather latency hides under the remaining S^T stream.
            # f32 transposes (PSUM banks are f32-native); the SBUF copy
            # casts to bf16 for the AllGather.
            otr = ps_s_pool.tile(
                [128, 512], F32, tag="pss", name=f"{R}otr{m}_{half}"
            )
            for qt in range(4):
                nc.tensor.transpose(
                    otr[:, qt * 128 : (qt + 1) * 128],
                    o_sb[m][:, qt, half * 128 : (half + 1) * 128],
                    ident_r,
                )
            otr_sb = o_pool.tile(
                [128, 512], BF16, tag="otrsb", bufs=2, name=f"{R}otrsb{m}_{half}"
            )
            nc.vector.tensor_copy(otr_sb, otr)
            if env.get("dbg") and half == 1:
                nc.sync.dma_start(
                    out=env["dbg"]["osb"][m, :, :, :], in_=o_sb[m]
                )
            # scalar-engine DMA queue: keeps the gather path ordered among
            # itself but independent of the (long) nat-load sync queue
            nc.scalar.dma_start(
                out=ag_in[m, half * 128 : (half + 1) * 128, :].rearrange(
                    "(c p) f -> p c f", p=128
                ),
                in_=otr_sb,
            )
            if single_core:
                for rr in range(4):
                    nc.scalar.dma_start(
                        out=ag_out[m, half, rr, :, :],
                        in_=ag_in[m, half * 128 : (half + 1) * 128, :],
                    )
            else:
                nc.gpsimd.collective_compute(
                    "AllGather",
                    bass.mybir.AluOpType.bypass,
                    replica_groups=[[0, 1, 2, 3], [4, 5, 6, 7]],
                    ins=[ag_in[m, half * 128 : (half + 1) * 128, :].opt()],
                    outs=[ag_out[m, half, :, :, :].opt()],
                )

        og = {}
        osb = {}

        def og_load(m):
            # og ct blocks: gathered channel block ct = cc*2 + half
            # (core cc's heads, pair half) -> og[:, ct, :]
            og[m] = og_pool.tile([128, CT, 512], BF16, tag="og", name=f"{R}og{m}")
            for half in range(2):
                nc.scalar.dma_start(
                    out=og[m][:, :, :].rearrange(
                        "p (cc hh) f -> p hh cc f", hh=2
                    )[:, half, :, :],
                    in_=ag_out[m, half, :, :, :].rearrange(
                        "cc p f -> p cc f"
                    ),
                )
            osb[m] = osb_pool.tile([128, 2, 512], F32, tag="osb", name=f"{R}osb{m}")

        def out_proj(m, ot):
            po = ps_s_pool.tile([128, 512], F32, tag="pss", name=f"{R}po{m}_{ot}")
            for ct in range(CT):
                nc.tensor.matmul(
                    po,
                    wo_bf[:, ct, ot * 128 : (ot + 1) * 128],
                    og[m][:, ct, :],
                    start=(ct == 0),
                    stop=(ct == CT - 1),
                )
            nc.vector.tensor_scalar_add(
                osb[m][:, ot, :], po, bo_sb[:, ot : ot + 1]
            )
            if ot == 1:
                nc.scalar.dma_start(
                    out=env["outT_d"][:, m * 512 : (m + 1) * 512].rearrange(
                        "(o p) f -> p o f", p=128
                    ),
                    in_=osb[m],
                )
                og.pop(m)
                osb.pop(m)

        # ---- fill-queue scheduler ----
        emit_log = []
        log_on = bool(int(os.environ.get("KERNEL_EMIT_LOG", "0")))
        fill = []
        fill_hi = []  # popped first: Q chains (unblock the next m's S^T)
        deferred = []  # (ready_slot, item): held until slot counter passes
        slot = [0]
        deficit = [0.0]

        def release_deferred():
            while deferred and deferred[0][0] <= slot[0]:
                fill.append(deferred.pop(0)[1])

        def pop_one():
            item = (fill_hi or fill).pop(0)
            if len(item) == 3:
                ns, fn, label = item
            else:
                (ns, fn), label = item, getattr(item[1], "__name__", "?")
            if log_on:
                emit_log.append(f"pop:{label}")
            fn()
            return ns

        def pop_fill(budget):
            deficit[0] = min(deficit[0] + budget, 4 * FILL_NS)
            while (fill or fill_hi) and deficit[0] > 0:
                deficit[0] -= pop_one()

        kq_emitted = set()

        def ensure_kq(t, c):
            if (t, c) in kq_emitted:
                return
            kq_emitted.add((t, c))
            tr_chunk(t, c, 0)
            tr_chunk(t, c, 1)
            kq_proj(t, c, 0)
            kq_proj(t, c, 1)

        # prefetch the first two chunks (q0 for Qproj(m0), k0 for S^T nt0)
        # BEFORE the weight DMAs — the first transposes are the critical
        # path; weights aren't read until the first projection (~7us in)
        load_next()
        load_next()

        w_sb = {}
        for tname, wd in (("k", "wkT_d"), ("q", "wqT_d"), ("v", "wvT_d")):
            w_sb[tname] = w_pool.tile([128, CT, PC], F32R, name=f"{R}w{tname}")
            nc.gpsimd.dma_start(
                out=w_sb[tname],
                in_=env[wd][:, :].rearrange("(ct p) c -> p ct c", p=128),
            )
        wo_f32 = nat_pool.tile(
            [128, CT, PC], F32, tag="wof", bufs=1, name=f"{R}wof"
        )
        nc.gpsimd.dma_start(
            out=wo_f32,
            in_=env["woT_d"][:, :].rearrange("(ct p) c -> p ct c", p=128),
        )
        wo_bf = w_pool.tile([128, CT, PC], BF16, name=f"{R}wob")
        nc.vector.tensor_copy(wo_bf, wo_f32)

        v_enqueued = [False]

        def enqueue_v_chain():
            if v_enqueued[0]:
                return
            v_enqueued[0] = True
            for c in range(4):
                fill.append(
                    (900.0, lambda c=c: tr_chunk("v", c, 0), f"Vtr0:{c}")
                )
                fill.append(
                    (900.0, lambda c=c: tr_chunk("v", c, 1), f"Vtr1:{c}")
                )
                for k in range(4):
                    fill.append(
                        (900.0, lambda n=c * 4 + k: v_proj(n),
                         f"Vproj:{c * 4 + k}")
                    )

        av_pending = []

        def flush_av():
            while av_pending and vproj_done[0] > av_pending[0][2]:
                mm, pp, ntile = av_pending.pop(0)
                fill.append(
                    (450.0, lambda a=mm, b=pp, c=ntile: av_quantum(a, b, c),
                     f"AV:{mm}{pp}.{ntile}")
                )

        # ---- main S^T driver ----
        for m in range(MT):
            # finish any in-flight pre-emitted Q chain for this m first
            while m > 0 and ("q", m) not in kq_emitted and (fill or fill_hi):
                flush_av()
                pop_one()
            ensure_kq("q", m)
            for p in range(2):
                for ntile in range(NT):
                    ensure_kq("k", ntile // 4)
                    if m == 0 and p == 0 and ntile == 4:
                        enqueue_v_chain()
                    if m < MT - 1 and p == 1 and ntile == 4:
                        # pre-emit next m's Q chain (high priority) so the
                        # m boundary doesn't stall the S^T/exp stream
                        nm = m + 1
                        fill_hi.append(
                            (900.0, lambda c=nm: tr_chunk("q", c, 0),
                             f"Qtr0:{nm}")
                        )
                        fill_hi.append(
                            (900.0, lambda c=nm: tr_chunk("q", c, 1),
                             f"Qtr1:{nm}")
                        )
                        fill_hi.append(
                            (900.0, lambda c=nm: kq_proj("q", c, 0),
                             f"Qproj0:{nm}")
                        )

                        def q_done(c=nm):
                            kq_proj("q", c, 1)
                            kq_emitted.add(("q", c))

                        fill_hi.append((900.0, q_done, f"Qproj1:{nm}"))
                    # at-pool headroom: force-drain fill before exp would
                    # block the Act engine on a full at pool (deadlock via
                    # the in-order PE stream otherwise)
                    while len(at_tiles) >= at_bufs_n - 1 and (fill or fill_hi):
                        flush_av()
                        pop_one()
                    if log_on:
                        emit_log.append(f"ST:{m}{p}.{ntile}")
                    st_quantum(m, p, ntile)
                    slot[0] += 1
                    release_deferred()
                    av_pending.append((m, p, ntile))
                    flush_av()
                    pop_fill(FILL_NS)

                def block_tail(m=m, p=p):
                    norm(m, p)
                    otr_ag(m, p)
                    if p == 1:
                        og_load(m)
                        # hold the out-projection back ~20 S^T slots so the
                        # in-order PE stream doesn't hit it before the
                        # AllGather + og DMA have landed
                        deferred.append(
                            (slot[0] + 20,
                             (1750.0, lambda: out_proj(m, 0), f"oproj0:{m}"))
                        )
                        deferred.append(
                            (slot[0] + 20,
                             (1750.0, lambda: out_proj(m, 1), f"oproj1:{m}"))
                        )
                fill.append((500.0, block_tail, f"tail:{m}{p}"))

        # drain
        while av_pending or fill or fill_hi or deferred:
            slot[0] += 10_000
            release_deferred()
            flush_av()
            if fill or fill_hi:
                pop_one()
            else:
                assert not av_pending, "AV stuck without V projections"
        if log_on:
            print("EMIT ORDER:", " ".join(emit_log))

        if env.get("dbg"):
            d = env["dbg"]
            nc.sync.dma_start(out=d["kT"][:, :, :], in_=kT.bitcast(F32))
            nc.sync.dma_start(out=d["qT"][:, :, :], in_=qT.bitcast(F32))
            with tc.tile_pool(name=f"{R}dbgp", bufs=2) as dbgp:
                for h in range(HC):
                    vdump = dbgp.tile(
                        [128, NT, DH + 1], F32, tag="vd", bufs=1,
                        name=f"{R}vdump{h}",
                    )
                    nc.vector.tensor_copy(vdump, v2[:, h, :, :])
                    nc.sync.dma_start(out=d["v2"][:, h, :, :], in_=vdump)
                for m in range(MT):
                    gb = dbgp.tile([128, 2, 512], BF16, tag="gb", name=f"{R}gb{m}")
                    nc.scalar.dma_start(
                        out=gb,
                        in_=ag_in[m, :, :].rearrange("(c p) f -> p c f", p=128),
                    )
                    gf = dbgp.tile([128, 2, 512], F32, tag="gf", bufs=1, name=f"{R}gf{m}")
                    nc.vector.tensor_copy(gf, gb)
                    nc.scalar.dma_start(
                        out=d["agin"][m, :, :].rearrange("(c p) f -> p c f", p=128),
                        in_=gf,
                    )



def _make_in_maps(queries, keys, values, Wq, bq, Wk, bk, Wv, bv, Wo, bo):
    # bv folds through attention (softmax weights sum to 1) and the output
    # projection into an effective output bias; bk shifts every logit in a
    # row equally so softmax cancels it.
    bo_eff = bo + Wo @ bv
    c = np.ascontiguousarray
    in_maps = []
    for core in range(NCORES):
        b, r = core // 4, core % 4
        sl = slice(r * PC, (r + 1) * PC)
        in_maps.append(
            {
                "xq": c(queries[b]),
                "xk": c(keys[b]),
                "xv": c(values[b]),
                "wqT": c(Wq[sl, :].T),
                "wkT": c(Wk[sl, :].T),
                "wvT": c(Wv[sl, :].T),
                "woT": c(Wo.T[:, sl]),
                "bq": c(bq[sl].reshape(PC, 1)),
                "bo": c(bo_eff[sl].reshape(PC, 1)),
            }
        )
    return in_maps


def kernel(queries, keys, values, Wq, bq, Wk, bk, Wv, bv, Wo, bo, _trace=False):
    import concourse.bass_utils as bass_utils

    args = [queries, keys, values, Wq, bq, Wk, bk, Wv, bv, Wo, bo]
    args = [np.asarray(a, dtype=np.float32) for a in args]

    if "nc" not in _CACHE:
        _CACHE["nc"] = _build()
    nc = _CACHE["nc"]

    in_maps = _make_in_maps(*args)
    res = bass_utils.run_bass_kernel_spmd(
        nc, in_maps, core_ids=list(range(NCORES)), trace=_trace
    )
    _CACHE["last_result"] = res

    out = np.empty((B, M, D), dtype=np.float32)
    for core in range(NCORES):
        b, r = core // 4, core % 4
        out[b, :, r * PC : (r + 1) * PC] = res.results[core]["outT"].T
    return out
